# revision 76
# baseline (speedup 1.0000x reference)
"""Trainium2 Bass kernel for nn_BlockAttention (block-local attention with RoPE + gate).

Sharding: sequence-parallel over 8 cores. Flattened [B*S=8192, E] rows split into
8 contiguous shards of 1024 rows (4 blocks of 256; blocks never cross cores or
batch boundaries since 4096/256=16 blocks per batch, 4 per core).

Per-core dataflow (all matmuls fp16/bf16; fp8 is numerically unusable here --
the randn cos/sin tables make scores reach |28| so softmax is sharp and any
quantization on the q/k/v path shows up ~1:1 in the output):
  - xT [E, R] fp16 from host; q/k projections in transposed layout
    (features-on-partitions) via lhsT=W chunks; RoPE applied with
    host-prepared fp16 cos/sin tables (rotate = DMA partition-shift of a
    gpsimd-drained fp16 copy; sign folded into the sin table)
  - v and gate projections in NATURAL layout (rows-on-partitions) via
    lhsT=xT chunks; v gets a ones-column per head so the AV matmul emits
    softmax row-sums for free; gate is sigmoid()ed on drain
  - attention per (block, head): scores_T [k,q] fp16; exp on ScalarE -> es
    bf16 (bf16 exponent range handles exp(28) without max-subtraction);
    AV with lhsT=es -> av[q, 65] so row-sums land per-PARTITION: reciprocal
    on DVE, normalization folded into the ScalarE PSUM drain (per-partition
    scale AP) -> y fp16 natural
  - y *= sigmoid(gate) on DVE (fp16, 2x rate); PE transposes y -> yT;
    output projection in transposed layout; fp16 store, host un-transposes
"""
import sys

sys.path.insert(0, "/opt/trn_rl_repo")
import numpy as np

B, S, E = 2, 4096, 1024
H, D = 16, 64
BLK = 256
NCORES = 8
R = (B * S) // NCORES   # 1024 rows per core
NB = R // BLK           # 4 blocks per core
NCH = E // 128          # 8 feature chunks of 128
SCALE = 1.0 / np.sqrt(D)


def emit(tc, outs, ins):
    """Emit the per-core program. ins/outs are DRAM APs:
    ins  = [xT, wq, wk, wv, wg, wo, cos2, sin2]
    outs = [outT]
    """
    from contextlib import ExitStack
    import concourse.mybir as mybir
    from concourse.masks import make_identity

    F32 = mybir.dt.float32
    F16 = mybir.dt.float16
    BF16 = mybir.dt.bfloat16
    AF = mybir.ActivationFunctionType

    nc = tc.nc
    xT_d, wq_d, wk_d, wv_d, wg_d, wo_d, c2_d, s2_d = ins
    (outT_d,) = outs

    with ExitStack() as ctx:
        ep = ctx.enter_context
        consts = ep(tc.tile_pool(name="consts", bufs=1))
        big = ep(tc.tile_pool(name="big", bufs=1))
        work = ep(tc.tile_pool(name="work", bufs=2))
        wpool = work
        wnat = work
        rawp = work
        ropet = work
        espool = work
        smalls = work
        opool = work
        # PSUM: 8 banks
        big_ps = ep(tc.tile_pool(name="big_ps", bufs=2, space="PSUM"))
        s_ps_p = ep(tc.tile_pool(name="s_ps_p", bufs=3, space="PSUM"))
        av_ps_p = ep(tc.tile_pool(name="av_ps_p", bufs=2, space="PSUM"))
        tp_ps_p = ep(tc.tile_pool(name="tp_ps_p", bufs=1, space="PSUM"))

        # ---- constants / inputs resident in SBUF; spread the initial loads
        # across engine DMA queues so they run in parallel.
        # xt split over sync/scalar so chunks land in consumption order at
        # 2-queue bandwidth; wv + rope tables on gpsimd (idle until the
        # first rope shifts at ~45us).
        xt = big.tile([128, NCH, R], F16)
        for nh in range(2):
            for kc in range(NCH):
                (nc.sync if kc % 2 == 0 else nc.scalar).dma_start(
                    xt[:, kc, nh * 512:(nh + 1) * 512],
                    xT_d[kc * 128:(kc + 1) * 128, nh * 512:(nh + 1) * 512])
        wvb_half = []
        for nh in range(2):
            wvb = wnat.tile([128, NCH, 512], F16, tag="wv")
            for kc in range(NCH):
                nc.gpsimd.dma_start(
                    wvb[:, kc, :],
                    wv_d[kc * 128:(kc + 1) * 128, nh * 512:(nh + 1) * 512])
            wvb_half.append(wvb)
        c2 = consts.tile([128, R], F16)
        nc.sync.dma_start(c2[:], c2_d[:])
        s2 = consts.tile([128, R], F16)
        nc.scalar.dma_start(s2[:], s2_d[:])
        ident = consts.tile([128, 128], F16)
        make_identity(nc, ident[:])

        qT = big.tile([128, NCH, R], F16)
        kT = big.tile([128, NCH, R], F16)
        # v natural [row-chunk part, rc, head, 64+ones]: the ones column makes
        # each AV matmul also emit the softmax row-sums (output col 64).
        v = big.tile([128, NCH, H, 65], BF16)
        nc.vector.memset(v[:, :, :, 64], 1.0)
        sg = big.tile([128, NCH, R], F16)   # sigmoid(gate), TRANSPOSED
        y = big.tile([128, NCH, R], F16)    # attn out (normalized), natural
        yT = big.tile([128, NCH, R], F16)   # (y.T * sigmoid(gate)) for out-proj

        # ---- one transposed projection output chunk: 8-matmul psum group
        def mm_group(w, nh):
            ps = big_ps.tile([128, 512], F32, tag="big")
            for kc in range(NCH):
                nc.tensor.matmul(
                    ps[:],
                    w[:, kc, :],
                    xt[:, kc, nh * 512:(nh + 1) * 512],
                    start=(kc == 0),
                    stop=(kc == NCH - 1),
                )
            return ps

        def proj_chunk_rope(w, dst, mc):
            # RoPE: drain raw fp16 copy on GpSimd; rotate (partition swap
            # d <-> d+-32 within each head) rides the DMA engines as 4
            # partition-shifted SBUF->SBUF copies; sign lives in sin2.
            ps0 = mm_group(w, 0)
            ps1 = mm_group(w, 1)
            raw = rawp.tile([128, R], F16, tag="raw")
            # scalar takes one PSUM drain (plain Copy, no ACT table) to
            # unload the DVE, which carries the rest of the rope chain
            nc.scalar.activation(raw[:, 0:512], ps0[:], AF.Copy)
            nc.vector.tensor_copy(raw[:, 512:1024], ps1[:])
            t = ropet.tile([128, R], F16, tag="t")
            for h2 in (0, 64):
                nc.gpsimd.dma_start(t[h2:h2 + 32, :], raw[h2 + 32:h2 + 64, :])
                nc.gpsimd.dma_start(t[h2 + 32:h2 + 64, :], raw[h2:h2 + 32, :])
            dsl = dst[:, mc, :]
            nc.vector.tensor_mul(dsl, raw[:], c2[:])
            nc.vector.tensor_mul(t[:], t[:], s2[:])
            nc.vector.tensor_add(dsl, dsl, t[:])

        def proj_load_w(w_d, mc):
            w = wpool.tile([128, NCH, 128], F16, tag="w", bufs=3)
            src = w_d.rearrange("(kc p) m -> p kc m", p=128)
            nc.sync.dma_start(w[:], src[:, :, mc * 128:(mc + 1) * 128])
            return w

        # ---- natural-layout projection group: out rows rc; wb holds one
        # 512-wide feature half of the weight.
        def nat_group(wb, rc):
            ps = big_ps.tile([128, 512], F32, tag="big")
            for kc in range(NCH):
                nc.tensor.matmul(
                    ps[:],
                    xt[:, kc, rc * 128:(rc + 1) * 128],
                    wb[:, kc, :],
                    start=(kc == 0),
                    stop=(kc == NCH - 1),
                )
            return ps

        # ---- attention unit for (block b, head hi), software-pipelined in
        # three stages so the scalar/vector chain of unit i overlaps the PE
        # work of unit i+1.
        def attn_front(b, hi):
            c, pb = hi // 2, 64 * (hi % 2)
            sps = s_ps_p.tile([128, 512], F32, tag="s")
            for kph in range(2):
                nc.tensor.matmul(
                    sps[:, kph * 256:(kph + 1) * 256],
                    kT[pb:pb + 64, c,
                       b * 256 + kph * 128:b * 256 + (kph + 1) * 128],
                    qT[pb:pb + 64, c, b * 256:(b + 1) * 256],
                    start=True, stop=True,
                )
            es = espool.tile([128, 512], BF16, tag="es", bufs=4)
            nc.scalar.activation(es[:], sps[:], AF.Exp, scale=float(SCALE))
            return (b, hi, es)

        # av psum tiles are shared by the head pair (2c, 2c+1) of one chunk:
        # hh = hi % 2 selects the half; qh the 128-row q-half of the block.
        # Row-sums land in column 64 (ones-column of v). One bank per pair.
        av_tiles = {}

        def attn_mid(st):
            b, hi, es = st
            hh = hi % 2
            if hh == 0:
                avt = av_ps_p.tile([128, 2, 2, 65], F32, tag="av", name="avt")
                av_tiles[(b, hi // 2)] = avt
            avq = av_tiles[(b, hi // 2)]
            for qh in range(2):
                for kph in range(2):
                    nc.tensor.matmul(
                        avq[:, qh, hh, :],
                        es[:, kph * 256 + qh * 128:kph * 256 + (qh + 1) * 128],
                        v[:, 2 * b + kph, hi, :],
                        start=(kph == 0), stop=(kph == 1),
                    )
            return (b, hi)

        MULT = mybir.AluOpType.mult
        tpdone = {}
        tp_pending = []

        def attn_tail(st):
            # after both heads of the pair finished AV: reciprocal of the
            # row-sums (per-partition!), then normalize on the DVE drain via
            # tensor_scalar with a per-partition scale AP.
            b, hi = st
            if hi % 2 == 0:
                return
            c = hi // 2
            avq = av_tiles.pop((b, c))
            recips = smalls.tile([128, 2, 2], F32, tag="recips", bufs=3)
            nc.vector.reciprocal(recips[:], avq[:, :, :, 64])
            for qh in range(2):
                rc = 2 * b + qh
                for hh in range(2):
                    nc.vector.tensor_scalar(
                        y[:, rc, (2 * c + hh) * 64:(2 * c + hh + 1) * 64],
                        avq[:, qh, hh, 0:64],
                        recips[:, qh, hh:hh + 1], None, op0=MULT)
            # y columns for this head pair are final for rows (2b, 2b+1):
            # transpose the 128-wide feature chunk (fc == c); the gate
            # multiply is folded into the transpose drain (sg is transposed).
            rcg = b // 2
            tpdone[(c, rcg)] = tpdone.get((c, rcg), 0) + 1
            if tpdone[(c, rcg)] == 2:
                tp_pending.append((c, rcg))
            # emit an OLDER pending transpose burst (its DVE deps are long
            # done) so the PE never waits on a just-written y chunk.
            if len(tp_pending) > 2:
                emit_transpose(*tp_pending.pop(0))

        def emit_transpose(c, rcg):
            tp = tp_ps_p.tile([128, 4, 128], F16, tag="tp", name="tp")
            for r4 in range(4):
                nc.tensor.transpose(
                    tp[:, r4, :],
                    y[:, rcg * 4 + r4, c * 128:(c + 1) * 128],
                    ident[:])
            nc.vector.tensor_mul(
                yT[:, c, rcg * 512:(rcg + 1) * 512],
                tp[:].rearrange("p a b -> p (a b)"),
                sg[:, c, rcg * 512:(rcg + 1) * 512])

        def flush_transposes():
            while tp_pending:
                emit_transpose(*tp_pending.pop(0))

        # ================= emission schedule =================
        # Phase V: v projection (natural), 16 groups.
        for nh in range(2):
            wvb = wvb_half[nh]
            for rc in range(NCH):
                ps = nat_group(wvb, rc)
                # scalar engine is otherwise idle during the v phase
                nc.scalar.activation(
                    v[:, rc, 8 * nh:8 * nh + 8, 0:64],
                    ps[:].rearrange("p (h d) -> p h d", d=64), AF.Copy)

        # Gate projection (transposed, like q/k), sigmoid on drain. Gate
        # chunk c is emitted during qk chunk c+1 (see the main loop) -- late
        # enough to interleave, early enough for pair-c transpose drains,
        # and chunks 6,7 land after the qk loop where they hide the last
        # head pair's attention units.
        def gate_chunk(c):
            w = proj_load_w(wg_d, c)
            for nh in range(2):
                ps = mm_group(w, nh)
                nc.scalar.activation(sg[:, c, nh * 512:(nh + 1) * 512],
                                     ps[:], AF.Sigmoid)

        for c in range(NCH):
            gate_chunk(c)

        # wo resident (16KB/partition); loads trickle on the sync queue
        # during the qk phase (only tiny wq/wk chunk loads live there).
        wof = consts.tile([128, NCH, NCH, 128], F16)
        wo_src = wo_d.rearrange("(kc p) m -> p kc m", p=128)

        def outproj(oc, rhalf):
            ps = big_ps.tile([128, 512], F32, tag="big")
            for yc in range(NCH):
                nc.tensor.matmul(
                    ps[:],
                    wof[:, oc, yc, :],
                    yT[:, yc, rhalf * 512:(rhalf + 1) * 512],
                    start=(yc == 0),
                    stop=(yc == NCH - 1),
                )
            osb = opool.tile([128, 512], F16, tag="o", bufs=3)
            # alternate drain engines so neither scalar (exps) nor vector
            # (tail chains) serializes the output phase
            if oc % 2 == 0:
                nc.vector.tensor_copy(osb[:], ps[:])
            else:
                nc.scalar.activation(osb[:], ps[:], AF.Copy)
            out_q[0] = (out_q[0] + 1) % 3
            [nc.sync, nc.gpsimd, nc.scalar][out_q[0]].dma_start(
                outT_d[oc * 128:(oc + 1) * 128,
                       rhalf * 512:(rhalf + 1) * 512], osb[:])

        out_q = [0]

        # Phase QK + attention: per chunk c emit q/k projections + rope,
        # interleaved with the (pipelined) attention units of the previous
        # chunk's head pair; tails fold normalize+gate+transpose inline.
        pipe = []

        def push_unit(b, hi):
            pipe.append(attn_front(b, hi))
            if len(pipe) >= 2:
                st = attn_mid(pipe.pop(0))
                attn_tail(st)

        def flush_pipe():
            while pipe:
                st = attn_mid(pipe.pop(0))
                attn_tail(st)

        for c in range(NCH):
            w = proj_load_w(wq_d, c)
            proj_chunk_rope(w, qT, c)
            for b in (0, 1):
                if c > 0:
                    push_unit(b, 2 * (c - 1))
                    push_unit(b, 2 * (c - 1) + 1)
            w = proj_load_w(wk_d, c)
            proj_chunk_rope(w, kT, c)
            for b in (2, 3):
                if c > 0:
                    push_unit(b, 2 * (c - 1))
                    push_unit(b, 2 * (c - 1) + 1)
            if 1 <= c <= 2:
                for oc in range(4 * (c - 1), 4 * c):
                    nc.sync.dma_start(wof[:, oc, :, :],
                                      wo_src[:, :, oc * 128:(oc + 1) * 128])
        # last head pair: blocks 0,1 first so rhalf-0 of the out projection
        # can interleave with blocks 2,3's attention units; the deferred
        # last gate chunk provides PE cover for the trailing units.
        for b in (0, 1):
            push_unit(b, 14)
            push_unit(b, 15)
        push_unit(2, 14)
        push_unit(2, 15)
        while pipe:
            st = attn_mid(pipe.pop(0))
            attn_tail(st)
        flush_transposes()   # yT rhalf 0 now complete for every fc
        oc_next = iter(range(NCH))
        push_unit(3, 14)
        outproj(next(oc_next), 0)
        push_unit(3, 15)
        outproj(next(oc_next), 0)
        flush_pipe()
        flush_transposes()
        # interleave the remaining rhalf-0 groups with rhalf-1 so output
        # stores spread evenly over the phase instead of bunching at the end
        rest = [(oc, 0) for oc in oc_next]
        inter = []
        r1 = list(range(NCH))
        while rest or r1:
            if rest:
                inter.append(rest.pop(0))
            if r1:
                inter.append((r1.pop(0), 1))
        for oc, rhalf in inter:
            outproj(oc, rhalf)


def _build_nc():
    import concourse.bacc as bacc
    import concourse.mybir as mybir
    import concourse.tile as tile

    F16 = mybir.dt.float16
    nc = bacc.Bacc("TRN2", target_bir_lowering=False, debug=False)
    names_in = ["xT", "wq", "wk", "wv", "wg", "wo", "cos2", "sin2"]
    shapes_in = [[E, R], [E, E], [E, E], [E, E], [E, E], [E, E],
                 [128, R], [128, R]]
    dts_in = [F16] * 8
    ins = [
        nc.dram_tensor(n, s, dt, kind="ExternalInput").ap()
        for n, s, dt in zip(names_in, shapes_in, dts_in)
    ]
    outT = nc.dram_tensor("outT", [E, R], F16, kind="ExternalOutput").ap()
    with tile.TileContext(nc) as tc:
        emit(tc, [outT], ins)
    nc.compile()
    return nc


_NC_CACHE = {}


def host_prep(x, Wq, Wk, Wv, Wg, Wo, cos, sin):
    """Build the 8 per-core input maps."""
    x_flat = np.ascontiguousarray(x.reshape(B * S, E), dtype=np.float32)
    Wq = np.ascontiguousarray(Wq, dtype=np.float16)
    Wk = np.ascontiguousarray(Wk, dtype=np.float16)
    Wv = np.ascontiguousarray(Wv, dtype=np.float16)
    Wg = np.ascontiguousarray(Wg, dtype=np.float16)
    Wo = np.ascontiguousarray(Wo, dtype=np.float16)
    cos = np.asarray(cos, dtype=np.float32)
    sin = np.asarray(sin, dtype=np.float32)
    sign = np.where(np.arange(D) < D // 2, -1.0, 1.0).astype(np.float32)

    in_maps = []
    for cix in range(NCORES):
        rows = slice(cix * R, (cix + 1) * R)
        xT = np.ascontiguousarray(x_flat[rows].T.astype(np.float16))
        seq = (cix * R + np.arange(R)) % S
        cS = cos[seq]            # [R, D]
        sS = sin[seq] * sign     # [R, D] signed
        c2 = np.ascontiguousarray(np.tile(cS.T, (2, 1)).astype(np.float16))
        s2 = np.ascontiguousarray(np.tile(sS.T, (2, 1)).astype(np.float16))
        in_maps.append({
            "xT": xT, "wq": Wq, "wk": Wk, "wv": Wv, "wg": Wg, "wo": Wo,
            "cos2": c2, "sin2": s2,
        })
    return in_maps


def kernel_traced(x, Wq, Wk, Wv, Wg, Wo, cos, sin, block_size, trace=False,
                  **run_kwargs):
    assert int(block_size) == BLK
    from concourse import bass_utils

    if "nc" not in _NC_CACHE:
        _NC_CACHE["nc"] = _build_nc()
    nc = _NC_CACHE["nc"]

    in_maps = host_prep(x, Wq, Wk, Wv, Wg, Wo, cos, sin)
    res = bass_utils.run_bass_kernel_spmd(
        nc, in_maps, core_ids=list(range(NCORES)), trace=trace, **run_kwargs)
    out_flat = np.empty((B * S, E), dtype=np.float32)
    for cix in range(NCORES):
        out_flat[cix * R:(cix + 1) * R] = res.results[cix]["outT"].T
    return out_flat.reshape(B, S, E), res


def kernel(x, Wq, Wk, Wv, Wg, Wo, cos, sin, block_size):
    return kernel_traced(x, Wq, Wk, Wv, Wg, Wo, cos, sin, block_size)[0]


# revision 77
# speedup vs baseline: 1.0121x; 1.0121x over previous
"""Trainium2 Bass kernel for nn_BlockAttention (block-local attention with RoPE + gate).

Sharding: sequence-parallel over 8 cores. Flattened [B*S=8192, E] rows split into
8 contiguous shards of 1024 rows (4 blocks of 256; blocks never cross cores or
batch boundaries since 4096/256=16 blocks per batch, 4 per core).

Per-core dataflow (all matmuls fp16/bf16; fp8 is numerically unusable here --
the randn cos/sin tables make scores reach |28| so softmax is sharp and any
quantization on the q/k/v path shows up ~1:1 in the output):
  - xT [E, R] fp16 from host; q/k projections in transposed layout
    (features-on-partitions) via lhsT=W chunks; RoPE applied with
    host-prepared fp16 cos/sin tables (rotate = DMA partition-shift of a
    gpsimd-drained fp16 copy; sign folded into the sin table)
  - v and gate projections in NATURAL layout (rows-on-partitions) via
    lhsT=xT chunks; v gets a ones-column per head so the AV matmul emits
    softmax row-sums for free; gate is sigmoid()ed on drain
  - attention per (block, head): scores_T [k,q] fp16; exp on ScalarE -> es
    bf16 (bf16 exponent range handles exp(28) without max-subtraction);
    AV with lhsT=es -> av[q, 65] so row-sums land per-PARTITION: reciprocal
    on DVE, normalization folded into the ScalarE PSUM drain (per-partition
    scale AP) -> y fp16 natural
  - y *= sigmoid(gate) on DVE (fp16, 2x rate); PE transposes y -> yT;
    output projection in transposed layout; fp16 store, host un-transposes
"""
import sys

sys.path.insert(0, "/opt/trn_rl_repo")
import numpy as np

B, S, E = 2, 4096, 1024
H, D = 16, 64
BLK = 256
NCORES = 8
R = (B * S) // NCORES   # 1024 rows per core
NB = R // BLK           # 4 blocks per core
NCH = E // 128          # 8 feature chunks of 128
SCALE = 1.0 / np.sqrt(D)


def emit(tc, outs, ins):
    """Emit the per-core program. ins/outs are DRAM APs:
    ins  = [xT, wq, wk, wv, wg, wo, cos2, sin2]
    outs = [outT]
    """
    from contextlib import ExitStack
    import concourse.mybir as mybir
    from concourse.masks import make_identity

    F32 = mybir.dt.float32
    F16 = mybir.dt.float16
    BF16 = mybir.dt.bfloat16
    AF = mybir.ActivationFunctionType

    nc = tc.nc
    xT_d, wq_d, wk_d, wv_d, wg_d, wo_d, c2_d, s2_d = ins
    (outT_d,) = outs

    with ExitStack() as ctx:
        ep = ctx.enter_context
        consts = ep(tc.tile_pool(name="consts", bufs=1))
        big = ep(tc.tile_pool(name="big", bufs=1))
        work = ep(tc.tile_pool(name="work", bufs=2))
        wpool = work
        wnat = work
        rawp = work
        ropet = work
        espool = work
        smalls = work
        opool = work
        # PSUM: 8 banks
        big_ps = ep(tc.tile_pool(name="big_ps", bufs=2, space="PSUM"))
        s_ps_p = ep(tc.tile_pool(name="s_ps_p", bufs=3, space="PSUM"))
        av_ps_p = ep(tc.tile_pool(name="av_ps_p", bufs=2, space="PSUM"))
        tp_ps_p = ep(tc.tile_pool(name="tp_ps_p", bufs=1, space="PSUM"))

        # ---- constants / inputs resident in SBUF; spread the initial loads
        # across engine DMA queues so they run in parallel.
        # xt split over sync/scalar so chunks land in consumption order at
        # 2-queue bandwidth; wv + rope tables on gpsimd (idle until the
        # first rope shifts at ~45us).
        xt = big.tile([128, NCH, R], F16)
        for nh in range(2):
            for kc in range(NCH):
                (nc.sync if kc % 2 == 0 else nc.scalar).dma_start(
                    xt[:, kc, nh * 512:(nh + 1) * 512],
                    xT_d[kc * 128:(kc + 1) * 128, nh * 512:(nh + 1) * 512])
        wvb_half = []
        for nh in range(2):
            wvb = wnat.tile([128, NCH, 512], F16, tag="wv")
            for kc in range(NCH):
                nc.gpsimd.dma_start(
                    wvb[:, kc, :],
                    wv_d[kc * 128:(kc + 1) * 128, nh * 512:(nh + 1) * 512])
            wvb_half.append(wvb)
        c2 = consts.tile([128, R], F16)
        nc.sync.dma_start(c2[:], c2_d[:])
        s2 = consts.tile([128, R], F16)
        nc.scalar.dma_start(s2[:], s2_d[:])
        ident = consts.tile([128, 128], F16)
        make_identity(nc, ident[:])

        qT = big.tile([128, NCH, R], F16)
        kT = big.tile([128, NCH, R], F16)
        # v natural [row-chunk part, rc, head, 64+ones]: the ones column makes
        # each AV matmul also emit the softmax row-sums (output col 64).
        v = big.tile([128, NCH, H, 65], BF16)
        nc.vector.memset(v[:, :, :, 64], 1.0)
        sg = big.tile([128, NCH, R], F16)   # sigmoid(gate), TRANSPOSED
        y = big.tile([128, NCH, R], F16)    # attn out (normalized), natural
        yT = big.tile([128, NCH, R], F16)   # (y.T * sigmoid(gate)) for out-proj

        # ---- one transposed projection output chunk: 8-matmul psum group
        def mm_group(w, nh):
            ps = big_ps.tile([128, 512], F32, tag="big")
            for kc in range(NCH):
                nc.tensor.matmul(
                    ps[:],
                    w[:, kc, :],
                    xt[:, kc, nh * 512:(nh + 1) * 512],
                    start=(kc == 0),
                    stop=(kc == NCH - 1),
                )
            return ps

        def proj_chunk_rope(w, dst, mc):
            # RoPE: drain raw fp16 copy on GpSimd; rotate (partition swap
            # d <-> d+-32 within each head) rides the DMA engines as 4
            # partition-shifted SBUF->SBUF copies; sign lives in sin2.
            ps0 = mm_group(w, 0)
            ps1 = mm_group(w, 1)
            raw = rawp.tile([128, R], F16, tag="raw")
            # scalar takes one PSUM drain (plain Copy, no ACT table) to
            # unload the DVE, which carries the rest of the rope chain
            nc.scalar.activation(raw[:, 0:512], ps0[:], AF.Copy)
            nc.vector.tensor_copy(raw[:, 512:1024], ps1[:])
            t = ropet.tile([128, R], F16, tag="t")
            for h2 in (0, 64):
                nc.gpsimd.dma_start(t[h2:h2 + 32, :], raw[h2 + 32:h2 + 64, :])
                nc.gpsimd.dma_start(t[h2 + 32:h2 + 64, :], raw[h2:h2 + 32, :])
            dsl = dst[:, mc, :]
            nc.vector.tensor_mul(dsl, raw[:], c2[:])
            nc.vector.tensor_mul(t[:], t[:], s2[:])
            nc.vector.tensor_add(dsl, dsl, t[:])

        def proj_load_w(w_d, mc):
            w = wpool.tile([128, NCH, 128], F16, tag="w", bufs=3)
            src = w_d.rearrange("(kc p) m -> p kc m", p=128)
            nc.sync.dma_start(w[:], src[:, :, mc * 128:(mc + 1) * 128])
            return w

        # ---- natural-layout projection group: out rows rc; wb holds one
        # 512-wide feature half of the weight.
        def nat_group(wb, rc):
            ps = big_ps.tile([128, 512], F32, tag="big")
            for kc in range(NCH):
                nc.tensor.matmul(
                    ps[:],
                    xt[:, kc, rc * 128:(rc + 1) * 128],
                    wb[:, kc, :],
                    start=(kc == 0),
                    stop=(kc == NCH - 1),
                )
            return ps

        # ---- attention unit for (block b, head hi), software-pipelined in
        # three stages so the scalar/vector chain of unit i overlaps the PE
        # work of unit i+1.
        def attn_front(b, hi):
            c, pb = hi // 2, 64 * (hi % 2)
            sps = s_ps_p.tile([128, 512], F32, tag="s")
            for kph in range(2):
                nc.tensor.matmul(
                    sps[:, kph * 256:(kph + 1) * 256],
                    kT[pb:pb + 64, c,
                       b * 256 + kph * 128:b * 256 + (kph + 1) * 128],
                    qT[pb:pb + 64, c, b * 256:(b + 1) * 256],
                    start=True, stop=True,
                )
            es = espool.tile([128, 512], BF16, tag="es", bufs=4)
            nc.scalar.activation(es[:], sps[:], AF.Exp, scale=float(SCALE))
            return (b, hi, es)

        # av psum tiles are shared by the head pair (2c, 2c+1) of one chunk:
        # hh = hi % 2 selects the half; qh the 128-row q-half of the block.
        # Row-sums land in column 64 (ones-column of v). One bank per pair.
        av_tiles = {}

        def attn_mid(st):
            b, hi, es = st
            hh = hi % 2
            if hh == 0:
                avt = av_ps_p.tile([128, 2, 2, 65], F32, tag="av", name="avt")
                av_tiles[(b, hi // 2)] = avt
            avq = av_tiles[(b, hi // 2)]
            for qh in range(2):
                for kph in range(2):
                    nc.tensor.matmul(
                        avq[:, qh, hh, :],
                        es[:, kph * 256 + qh * 128:kph * 256 + (qh + 1) * 128],
                        v[:, 2 * b + kph, hi, :],
                        start=(kph == 0), stop=(kph == 1),
                    )
            return (b, hi)

        MULT = mybir.AluOpType.mult
        tpdone = {}
        tp_pending = []

        def attn_tail(st):
            # after both heads of the pair finished AV: reciprocal of the
            # row-sums (per-partition!), then normalize on the DVE drain via
            # tensor_scalar with a per-partition scale AP.
            b, hi = st
            if hi % 2 == 0:
                return
            c = hi // 2
            avq = av_tiles.pop((b, c))
            recips = smalls.tile([128, 2, 2], F32, tag="recips", bufs=3)
            nc.vector.reciprocal(recips[:], avq[:, :, :, 64])
            for qh in range(2):
                rc = 2 * b + qh
                for hh in range(2):
                    nc.vector.tensor_scalar(
                        y[:, rc, (2 * c + hh) * 64:(2 * c + hh + 1) * 64],
                        avq[:, qh, hh, 0:64],
                        recips[:, qh, hh:hh + 1], None, op0=MULT)
            # y columns for this head pair are final for rows (2b, 2b+1):
            # transpose the 128-wide feature chunk (fc == c); the gate
            # multiply is folded into the transpose drain (sg is transposed).
            rcg = b // 2
            tpdone[(c, rcg)] = tpdone.get((c, rcg), 0) + 1
            if tpdone[(c, rcg)] == 2:
                tp_pending.append((c, rcg))
            # emit an OLDER pending transpose burst (its DVE deps are long
            # done) so the PE never waits on a just-written y chunk.
            if len(tp_pending) > 2:
                emit_transpose(*tp_pending.pop(0))

        def emit_transpose(c, rcg):
            tp = tp_ps_p.tile([128, 4, 128], F16, tag="tp", name="tp")
            for r4 in range(4):
                nc.tensor.transpose(
                    tp[:, r4, :],
                    y[:, rcg * 4 + r4, c * 128:(c + 1) * 128],
                    ident[:])
            nc.vector.tensor_mul(
                yT[:, c, rcg * 512:(rcg + 1) * 512],
                tp[:].rearrange("p a b -> p (a b)"),
                sg[:, c, rcg * 512:(rcg + 1) * 512])

        def flush_transposes():
            while tp_pending:
                emit_transpose(*tp_pending.pop(0))

        # ================= emission schedule =================
        # Phase V: v projection (natural), 16 groups.
        for nh in range(2):
            wvb = wvb_half[nh]
            for rc in range(NCH):
                ps = nat_group(wvb, rc)
                # scalar engine is otherwise idle during the v phase
                nc.scalar.activation(
                    v[:, rc, 8 * nh:8 * nh + 8, 0:64],
                    ps[:].rearrange("p (h d) -> p h d", d=64), AF.Copy)

        # Gate projection (transposed, like q/k), sigmoid on drain. Gate
        # chunk c is emitted during qk chunk c+1 (see the main loop) -- late
        # enough to interleave, early enough for pair-c transpose drains,
        # and chunks 6,7 land after the qk loop where they hide the last
        # head pair's attention units.
        def gate_chunk(c):
            w = proj_load_w(wg_d, c)
            for nh in range(2):
                ps = mm_group(w, nh)
                nc.scalar.activation(sg[:, c, nh * 512:(nh + 1) * 512],
                                     ps[:], AF.Sigmoid)

        for c in range(NCH):
            gate_chunk(c)

        # wo resident (16KB/partition); loads trickle on the sync queue
        # during the qk phase (only tiny wq/wk chunk loads live there).
        wof = consts.tile([128, NCH, NCH, 128], F16)
        wo_src = wo_d.rearrange("(kc p) m -> p kc m", p=128)

        def outproj(oc, rhalf):
            ps = big_ps.tile([128, 512], F32, tag="big")
            for yc in range(NCH):
                nc.tensor.matmul(
                    ps[:],
                    wof[:, oc, yc, :],
                    yT[:, yc, rhalf * 512:(rhalf + 1) * 512],
                    start=(yc == 0),
                    stop=(yc == NCH - 1),
                )
            osb = opool.tile([128, 512], F16, tag="o", bufs=3)
            # rhalf 0 runs while the scalar engine is still doing the last
            # attention exps -- drain on vector there, scalar afterwards.
            if rhalf == 0:
                nc.vector.tensor_copy(osb[:], ps[:])
            else:
                nc.scalar.activation(osb[:], ps[:], AF.Copy)
            out_q[0] = (out_q[0] + 1) % 3
            [nc.sync, nc.gpsimd, nc.scalar][out_q[0]].dma_start(
                outT_d[oc * 128:(oc + 1) * 128,
                       rhalf * 512:(rhalf + 1) * 512], osb[:])

        out_q = [0]

        # Phase QK + attention: per chunk c emit q/k projections + rope,
        # interleaved with the (pipelined) attention units of the previous
        # chunk's head pair; tails fold normalize+gate+transpose inline.
        pipe = []

        def push_unit(b, hi):
            pipe.append(attn_front(b, hi))
            if len(pipe) >= 2:
                st = attn_mid(pipe.pop(0))
                attn_tail(st)

        def flush_pipe():
            while pipe:
                st = attn_mid(pipe.pop(0))
                attn_tail(st)

        for c in range(NCH):
            w = proj_load_w(wq_d, c)
            proj_chunk_rope(w, qT, c)
            for b in (0, 1):
                if c > 0:
                    push_unit(b, 2 * (c - 1))
                    push_unit(b, 2 * (c - 1) + 1)
            w = proj_load_w(wk_d, c)
            proj_chunk_rope(w, kT, c)
            for b in (2, 3):
                if c > 0:
                    push_unit(b, 2 * (c - 1))
                    push_unit(b, 2 * (c - 1) + 1)
            if 1 <= c <= 2:
                for oc in range(4 * (c - 1), 4 * c):
                    nc.sync.dma_start(wof[:, oc, :, :],
                                      wo_src[:, :, oc * 128:(oc + 1) * 128])
        # last head pair: blocks 0,1 first so rhalf-0 of the out projection
        # can interleave with blocks 2,3's attention units; the deferred
        # last gate chunk provides PE cover for the trailing units.
        for b in (0, 1):
            push_unit(b, 14)
            push_unit(b, 15)
        push_unit(2, 14)
        push_unit(2, 15)
        while pipe:
            st = attn_mid(pipe.pop(0))
            attn_tail(st)
        flush_transposes()   # yT rhalf 0 now complete for every fc
        oc_next = iter(range(NCH))
        push_unit(3, 14)
        outproj(next(oc_next), 0)
        push_unit(3, 15)
        outproj(next(oc_next), 0)
        flush_pipe()
        flush_transposes()
        # interleave the remaining rhalf-0 groups with rhalf-1 so output
        # stores spread evenly over the phase instead of bunching at the end
        rest = [(oc, 0) for oc in oc_next]
        inter = []
        r1 = list(range(NCH))
        while rest or r1:
            if rest:
                inter.append(rest.pop(0))
            if r1:
                inter.append((r1.pop(0), 1))
        for oc, rhalf in inter:
            outproj(oc, rhalf)


def _build_nc():
    import concourse.bacc as bacc
    import concourse.mybir as mybir
    import concourse.tile as tile

    F16 = mybir.dt.float16
    nc = bacc.Bacc("TRN2", target_bir_lowering=False, debug=False)
    names_in = ["xT", "wq", "wk", "wv", "wg", "wo", "cos2", "sin2"]
    shapes_in = [[E, R], [E, E], [E, E], [E, E], [E, E], [E, E],
                 [128, R], [128, R]]
    dts_in = [F16] * 8
    ins = [
        nc.dram_tensor(n, s, dt, kind="ExternalInput").ap()
        for n, s, dt in zip(names_in, shapes_in, dts_in)
    ]
    outT = nc.dram_tensor("outT", [E, R], F16, kind="ExternalOutput").ap()
    with tile.TileContext(nc) as tc:
        emit(tc, [outT], ins)
    nc.compile()
    return nc


_NC_CACHE = {}


def host_prep(x, Wq, Wk, Wv, Wg, Wo, cos, sin):
    """Build the 8 per-core input maps."""
    x_flat = np.ascontiguousarray(x.reshape(B * S, E), dtype=np.float32)
    Wq = np.ascontiguousarray(Wq, dtype=np.float16)
    Wk = np.ascontiguousarray(Wk, dtype=np.float16)
    Wv = np.ascontiguousarray(Wv, dtype=np.float16)
    Wg = np.ascontiguousarray(Wg, dtype=np.float16)
    Wo = np.ascontiguousarray(Wo, dtype=np.float16)
    cos = np.asarray(cos, dtype=np.float32)
    sin = np.asarray(sin, dtype=np.float32)
    sign = np.where(np.arange(D) < D // 2, -1.0, 1.0).astype(np.float32)

    in_maps = []
    for cix in range(NCORES):
        rows = slice(cix * R, (cix + 1) * R)
        xT = np.ascontiguousarray(x_flat[rows].T.astype(np.float16))
        seq = (cix * R + np.arange(R)) % S
        cS = cos[seq]            # [R, D]
        sS = sin[seq] * sign     # [R, D] signed
        c2 = np.ascontiguousarray(np.tile(cS.T, (2, 1)).astype(np.float16))
        s2 = np.ascontiguousarray(np.tile(sS.T, (2, 1)).astype(np.float16))
        in_maps.append({
            "xT": xT, "wq": Wq, "wk": Wk, "wv": Wv, "wg": Wg, "wo": Wo,
            "cos2": c2, "sin2": s2,
        })
    return in_maps


def kernel_traced(x, Wq, Wk, Wv, Wg, Wo, cos, sin, block_size, trace=False,
                  **run_kwargs):
    assert int(block_size) == BLK
    from concourse import bass_utils

    if "nc" not in _NC_CACHE:
        _NC_CACHE["nc"] = _build_nc()
    nc = _NC_CACHE["nc"]

    in_maps = host_prep(x, Wq, Wk, Wv, Wg, Wo, cos, sin)
    res = bass_utils.run_bass_kernel_spmd(
        nc, in_maps, core_ids=list(range(NCORES)), trace=trace, **run_kwargs)
    out_flat = np.empty((B * S, E), dtype=np.float32)
    for cix in range(NCORES):
        out_flat[cix * R:(cix + 1) * R] = res.results[cix]["outT"].T
    return out_flat.reshape(B, S, E), res


def kernel(x, Wq, Wk, Wv, Wg, Wo, cos, sin, block_size):
    return kernel_traced(x, Wq, Wk, Wv, Wg, Wo, cos, sin, block_size)[0]


# revision 78
# speedup vs baseline: 1.0361x; 1.0236x over previous
"""Trainium2 Bass kernel for nn_BlockAttention (block-local attention with RoPE + gate).

Sharding: sequence-parallel over 8 cores. Flattened [B*S=8192, E] rows split into
8 contiguous shards of 1024 rows (4 blocks of 256; blocks never cross cores or
batch boundaries since 4096/256=16 blocks per batch, 4 per core).

Per-core dataflow (all matmuls fp16/bf16; fp8 is numerically unusable here --
the randn cos/sin tables make scores reach |28| so softmax is sharp and any
quantization on the q/k/v path shows up ~1:1 in the output):
  - xT [E, R] fp16 from host; q/k projections in transposed layout
    (features-on-partitions) via lhsT=W chunks; RoPE applied with
    host-prepared fp16 cos/sin tables (rotate = DMA partition-shift of a
    gpsimd-drained fp16 copy; sign folded into the sin table)
  - v and gate projections in NATURAL layout (rows-on-partitions) via
    lhsT=xT chunks; v gets a ones-column per head so the AV matmul emits
    softmax row-sums for free; gate is sigmoid()ed on drain
  - attention per (block, head): scores_T [k,q] fp16; exp on ScalarE -> es
    bf16 (bf16 exponent range handles exp(28) without max-subtraction);
    AV with lhsT=es -> av[q, 65] so row-sums land per-PARTITION: reciprocal
    on DVE, normalization folded into the ScalarE PSUM drain (per-partition
    scale AP) -> y fp16 natural
  - y *= sigmoid(gate) on DVE (fp16, 2x rate); PE transposes y -> yT;
    output projection in transposed layout; fp16 store, host un-transposes
"""
import sys

sys.path.insert(0, "/opt/trn_rl_repo")
import numpy as np

B, S, E = 2, 4096, 1024
H, D = 16, 64
BLK = 256
NCORES = 8
R = (B * S) // NCORES   # 1024 rows per core
NB = R // BLK           # 4 blocks per core
NCH = E // 128          # 8 feature chunks of 128
SCALE = 1.0 / np.sqrt(D)


def emit(tc, outs, ins):
    """Emit the per-core program. ins/outs are DRAM APs:
    ins  = [xT, wq, wk, wv, wg, wo, cos2, sin2]
    outs = [outT]
    """
    from contextlib import ExitStack
    import concourse.mybir as mybir
    from concourse.masks import make_identity

    F32 = mybir.dt.float32
    F16 = mybir.dt.float16
    BF16 = mybir.dt.bfloat16
    AF = mybir.ActivationFunctionType

    nc = tc.nc
    xT_d, wq_d, wk_d, wv_d, wg_d, wo_d, c2_d, s2_d = ins
    (outT_d,) = outs

    with ExitStack() as ctx:
        ep = ctx.enter_context
        consts = ep(tc.tile_pool(name="consts", bufs=1))
        big = ep(tc.tile_pool(name="big", bufs=1))
        work = ep(tc.tile_pool(name="work", bufs=2))
        wpool = work
        wnat = work
        rawp = work
        ropet = work
        espool = work
        smalls = work
        opool = work
        # PSUM: 8 banks
        big_ps = ep(tc.tile_pool(name="big_ps", bufs=2, space="PSUM"))
        s_ps_p = ep(tc.tile_pool(name="s_ps_p", bufs=3, space="PSUM"))
        av_ps_p = ep(tc.tile_pool(name="av_ps_p", bufs=2, space="PSUM"))
        tp_ps_p = ep(tc.tile_pool(name="tp_ps_p", bufs=1, space="PSUM"))

        # ---- constants / inputs resident in SBUF; spread the initial loads
        # across engine DMA queues so they run in parallel.
        # xt split over sync/scalar so chunks land in consumption order at
        # 2-queue bandwidth; wv + rope tables on gpsimd (idle until the
        # first rope shifts at ~45us).
        xt = big.tile([128, NCH, R], F16)
        for nh in range(2):
            for kc in range(NCH):
                (nc.sync if kc % 2 == 0 else nc.scalar).dma_start(
                    xt[:, kc, nh * 512:(nh + 1) * 512],
                    xT_d[kc * 128:(kc + 1) * 128, nh * 512:(nh + 1) * 512])
        wvb_half = []
        for nh in range(2):
            wvb = wnat.tile([128, NCH, 512], F16, tag="wv")
            for kc in range(NCH):
                nc.gpsimd.dma_start(
                    wvb[:, kc, :],
                    wv_d[kc * 128:(kc + 1) * 128, nh * 512:(nh + 1) * 512])
            wvb_half.append(wvb)
        c2 = consts.tile([128, R], F16)
        nc.sync.dma_start(c2[:], c2_d[:])
        s2 = consts.tile([128, R], F16)
        nc.scalar.dma_start(s2[:], s2_d[:])
        ident = consts.tile([128, 128], F16)
        make_identity(nc, ident[:])

        qT = big.tile([128, NCH, R], F16)
        kT = big.tile([128, NCH, R], F16)
        # v natural [row-chunk part, rc, head, 64+ones]: the ones column makes
        # each AV matmul also emit the softmax row-sums (output col 64).
        v = big.tile([128, NCH, H, 65], BF16)
        nc.vector.memset(v[:, :, :, 64], 1.0)
        sg = big.tile([128, NCH, R], F16)   # sigmoid(gate), TRANSPOSED
        y = big.tile([128, NCH, R], F16)    # attn out (normalized), natural
        yT = big.tile([128, NCH, R], F16)   # (y.T * sigmoid(gate)) for out-proj

        # ---- one transposed projection output chunk: 8-matmul psum group
        def mm_group(w, nh):
            ps = big_ps.tile([128, 512], F32, tag="big")
            for kc in range(NCH):
                nc.tensor.matmul(
                    ps[:],
                    w[:, kc, :],
                    xt[:, kc, nh * 512:(nh + 1) * 512],
                    start=(kc == 0),
                    stop=(kc == NCH - 1),
                )
            return ps

        def proj_chunk_rope(w, dst, mc):
            # RoPE: drain raw fp16 copy on GpSimd; rotate (partition swap
            # d <-> d+-32 within each head) rides the DMA engines as 4
            # partition-shifted SBUF->SBUF copies; sign lives in sin2.
            ps0 = mm_group(w, 0)
            ps1 = mm_group(w, 1)
            raw = rawp.tile([128, R], F16, tag="raw")
            # scalar takes one PSUM drain (plain Copy, no ACT table) to
            # unload the DVE, which carries the rest of the rope chain
            nc.scalar.activation(raw[:, 0:512], ps0[:], AF.Copy)
            nc.vector.tensor_copy(raw[:, 512:1024], ps1[:])
            t = ropet.tile([128, R], F16, tag="t")
            for h2 in (0, 64):
                nc.sync.dma_start(t[h2:h2 + 32, :], raw[h2 + 32:h2 + 64, :])
                nc.gpsimd.dma_start(t[h2 + 32:h2 + 64, :], raw[h2:h2 + 32, :])
            dsl = dst[:, mc, :]
            nc.vector.tensor_mul(dsl, raw[:], c2[:])
            nc.vector.tensor_mul(t[:], t[:], s2[:])
            nc.vector.tensor_add(dsl, dsl, t[:])

        def proj_load_w(w_d, mc):
            w = wpool.tile([128, NCH, 128], F16, tag="w", bufs=3)
            src = w_d.rearrange("(kc p) m -> p kc m", p=128)
            nc.sync.dma_start(w[:], src[:, :, mc * 128:(mc + 1) * 128])
            return w

        # ---- natural-layout projection group: out rows rc; wb holds one
        # 512-wide feature half of the weight.
        def nat_group(wb, rc):
            ps = big_ps.tile([128, 512], F32, tag="big")
            for kc in range(NCH):
                nc.tensor.matmul(
                    ps[:],
                    xt[:, kc, rc * 128:(rc + 1) * 128],
                    wb[:, kc, :],
                    start=(kc == 0),
                    stop=(kc == NCH - 1),
                )
            return ps

        # ---- attention unit for (block b, head hi), software-pipelined in
        # three stages so the scalar/vector chain of unit i overlaps the PE
        # work of unit i+1.
        def attn_front(b, hi):
            c, pb = hi // 2, 64 * (hi % 2)
            sps = s_ps_p.tile([128, 512], F32, tag="s")
            for kph in range(2):
                nc.tensor.matmul(
                    sps[:, kph * 256:(kph + 1) * 256],
                    kT[pb:pb + 64, c,
                       b * 256 + kph * 128:b * 256 + (kph + 1) * 128],
                    qT[pb:pb + 64, c, b * 256:(b + 1) * 256],
                    start=True, stop=True,
                )
            es = espool.tile([128, 512], BF16, tag="es", bufs=4)
            nc.scalar.activation(es[:], sps[:], AF.Exp, scale=float(SCALE))
            return (b, hi, es)

        # av psum tiles are shared by the head pair (2c, 2c+1) of one chunk:
        # hh = hi % 2 selects the half; qh the 128-row q-half of the block.
        # Row-sums land in column 64 (ones-column of v). One bank per pair.
        av_tiles = {}

        def attn_mid(st):
            b, hi, es = st
            hh = hi % 2
            if hh == 0:
                avt = av_ps_p.tile([128, 2, 2, 65], F32, tag="av", name="avt")
                av_tiles[(b, hi // 2)] = avt
            avq = av_tiles[(b, hi // 2)]
            for qh in range(2):
                for kph in range(2):
                    nc.tensor.matmul(
                        avq[:, qh, hh, :],
                        es[:, kph * 256 + qh * 128:kph * 256 + (qh + 1) * 128],
                        v[:, 2 * b + kph, hi, :],
                        start=(kph == 0), stop=(kph == 1),
                    )
            return (b, hi)

        MULT = mybir.AluOpType.mult
        tpdone = {}
        tp_pending = []

        def attn_tail(st):
            # after both heads of the pair finished AV: reciprocal of the
            # row-sums (per-partition!), then normalize on the DVE drain via
            # tensor_scalar with a per-partition scale AP.
            b, hi = st
            if hi % 2 == 0:
                return
            c = hi // 2
            avq = av_tiles.pop((b, c))
            recips = smalls.tile([128, 2, 2], F32, tag="recips", bufs=3)
            nc.vector.reciprocal(recips[:], avq[:, :, :, 64])
            for qh in range(2):
                rc = 2 * b + qh
                for hh in range(2):
                    nc.vector.tensor_scalar(
                        y[:, rc, (2 * c + hh) * 64:(2 * c + hh + 1) * 64],
                        avq[:, qh, hh, 0:64],
                        recips[:, qh, hh:hh + 1], None, op0=MULT)
            # y columns for this head pair are final for rows (2b, 2b+1):
            # transpose the 128-wide feature chunk (fc == c); the gate
            # multiply is folded into the transpose drain (sg is transposed).
            rcg = b // 2
            tpdone[(c, rcg)] = tpdone.get((c, rcg), 0) + 1
            if tpdone[(c, rcg)] == 2:
                tp_pending.append((c, rcg))
            # emit an OLDER pending transpose burst (its DVE deps are long
            # done) so the PE never waits on a just-written y chunk.
            if len(tp_pending) > 2:
                emit_transpose(*tp_pending.pop(0))

        def emit_transpose(c, rcg):
            tp = tp_ps_p.tile([128, 4, 128], F16, tag="tp", name="tp")
            for r4 in range(4):
                nc.tensor.transpose(
                    tp[:, r4, :],
                    y[:, rcg * 4 + r4, c * 128:(c + 1) * 128],
                    ident[:])
            nc.vector.tensor_mul(
                yT[:, c, rcg * 512:(rcg + 1) * 512],
                tp[:].rearrange("p a b -> p (a b)"),
                sg[:, c, rcg * 512:(rcg + 1) * 512])

        def flush_transposes():
            while tp_pending:
                emit_transpose(*tp_pending.pop(0))

        # ================= emission schedule =================
        # Phase V: v projection (natural), 16 groups.
        for nh in range(2):
            wvb = wvb_half[nh]
            for rc in range(NCH):
                ps = nat_group(wvb, rc)
                # scalar engine is otherwise idle during the v phase
                nc.scalar.activation(
                    v[:, rc, 8 * nh:8 * nh + 8, 0:64],
                    ps[:].rearrange("p (h d) -> p h d", d=64), AF.Copy)

        # Gate projection (transposed, like q/k), sigmoid on drain. Gate
        # chunk c is emitted during qk chunk c+1 (see the main loop) -- late
        # enough to interleave, early enough for pair-c transpose drains,
        # and chunks 6,7 land after the qk loop where they hide the last
        # head pair's attention units.
        def gate_chunk(c):
            w = proj_load_w(wg_d, c)
            for nh in range(2):
                ps = mm_group(w, nh)
                nc.scalar.activation(sg[:, c, nh * 512:(nh + 1) * 512],
                                     ps[:], AF.Sigmoid)

        for c in range(NCH):
            gate_chunk(c)

        # wo resident (16KB/partition); loads trickle on the sync queue
        # during the qk phase (only tiny wq/wk chunk loads live there).
        wof = consts.tile([128, NCH, NCH, 128], F16)
        wo_src = wo_d.rearrange("(kc p) m -> p kc m", p=128)

        def outproj(oc, rhalf):
            ps = big_ps.tile([128, 512], F32, tag="big")
            for yc in range(NCH):
                nc.tensor.matmul(
                    ps[:],
                    wof[:, oc, yc, :],
                    yT[:, yc, rhalf * 512:(rhalf + 1) * 512],
                    start=(yc == 0),
                    stop=(yc == NCH - 1),
                )
            osb = opool.tile([128, 512], F16, tag="o", bufs=3)
            # rhalf 0 runs while the scalar engine is still doing the last
            # attention exps -- drain on vector there, scalar afterwards.
            if rhalf == 0:
                nc.vector.tensor_copy(osb[:], ps[:])
            else:
                nc.scalar.activation(osb[:], ps[:], AF.Copy)
            out_q[0] = (out_q[0] + 1) % 3
            [nc.sync, nc.gpsimd, nc.scalar][out_q[0]].dma_start(
                outT_d[oc * 128:(oc + 1) * 128,
                       rhalf * 512:(rhalf + 1) * 512], osb[:])

        out_q = [0]

        # Phase QK + attention: per chunk c emit q/k projections + rope,
        # interleaved with the (pipelined) attention units of the previous
        # chunk's head pair; tails fold normalize+gate+transpose inline.
        pipe = []

        def push_unit(b, hi):
            pipe.append(attn_front(b, hi))
            if len(pipe) >= 2:
                st = attn_mid(pipe.pop(0))
                attn_tail(st)

        def flush_pipe():
            while pipe:
                st = attn_mid(pipe.pop(0))
                attn_tail(st)

        for c in range(NCH):
            w = proj_load_w(wq_d, c)
            proj_chunk_rope(w, qT, c)
            for b in (0, 1):
                if c > 0:
                    push_unit(b, 2 * (c - 1))
                    push_unit(b, 2 * (c - 1) + 1)
            w = proj_load_w(wk_d, c)
            proj_chunk_rope(w, kT, c)
            for b in (2, 3):
                if c > 0:
                    push_unit(b, 2 * (c - 1))
                    push_unit(b, 2 * (c - 1) + 1)
            if 1 <= c <= 2:
                for oc in range(4 * (c - 1), 4 * c):
                    nc.sync.dma_start(wof[:, oc, :, :],
                                      wo_src[:, :, oc * 128:(oc + 1) * 128])
        # last head pair: blocks 0,1 first so rhalf-0 of the out projection
        # can interleave with blocks 2,3's attention units; the deferred
        # last gate chunk provides PE cover for the trailing units.
        for b in (0, 1):
            push_unit(b, 14)
            push_unit(b, 15)
        push_unit(2, 14)
        push_unit(2, 15)
        while pipe:
            st = attn_mid(pipe.pop(0))
            attn_tail(st)
        flush_transposes()   # yT rhalf 0 now complete for every fc
        oc_next = iter(range(NCH))
        push_unit(3, 14)
        outproj(next(oc_next), 0)
        push_unit(3, 15)
        outproj(next(oc_next), 0)
        flush_pipe()
        flush_transposes()
        # interleave the remaining rhalf-0 groups with rhalf-1 so output
        # stores spread evenly over the phase instead of bunching at the end
        rest = [(oc, 0) for oc in oc_next]
        inter = []
        r1 = list(range(NCH))
        while rest or r1:
            if rest:
                inter.append(rest.pop(0))
            if r1:
                inter.append((r1.pop(0), 1))
        for oc, rhalf in inter:
            outproj(oc, rhalf)


def _build_nc():
    import concourse.bacc as bacc
    import concourse.mybir as mybir
    import concourse.tile as tile

    F16 = mybir.dt.float16
    nc = bacc.Bacc("TRN2", target_bir_lowering=False, debug=False)
    names_in = ["xT", "wq", "wk", "wv", "wg", "wo", "cos2", "sin2"]
    shapes_in = [[E, R], [E, E], [E, E], [E, E], [E, E], [E, E],
                 [128, R], [128, R]]
    dts_in = [F16] * 8
    ins = [
        nc.dram_tensor(n, s, dt, kind="ExternalInput").ap()
        for n, s, dt in zip(names_in, shapes_in, dts_in)
    ]
    outT = nc.dram_tensor("outT", [E, R], F16, kind="ExternalOutput").ap()
    with tile.TileContext(nc) as tc:
        emit(tc, [outT], ins)
    nc.compile()
    return nc


_NC_CACHE = {}


def host_prep(x, Wq, Wk, Wv, Wg, Wo, cos, sin):
    """Build the 8 per-core input maps."""
    x_flat = np.ascontiguousarray(x.reshape(B * S, E), dtype=np.float32)
    Wq = np.ascontiguousarray(Wq, dtype=np.float16)
    Wk = np.ascontiguousarray(Wk, dtype=np.float16)
    Wv = np.ascontiguousarray(Wv, dtype=np.float16)
    Wg = np.ascontiguousarray(Wg, dtype=np.float16)
    Wo = np.ascontiguousarray(Wo, dtype=np.float16)
    cos = np.asarray(cos, dtype=np.float32)
    sin = np.asarray(sin, dtype=np.float32)
    sign = np.where(np.arange(D) < D // 2, -1.0, 1.0).astype(np.float32)

    in_maps = []
    for cix in range(NCORES):
        rows = slice(cix * R, (cix + 1) * R)
        xT = np.ascontiguousarray(x_flat[rows].T.astype(np.float16))
        seq = (cix * R + np.arange(R)) % S
        cS = cos[seq]            # [R, D]
        sS = sin[seq] * sign     # [R, D] signed
        c2 = np.ascontiguousarray(np.tile(cS.T, (2, 1)).astype(np.float16))
        s2 = np.ascontiguousarray(np.tile(sS.T, (2, 1)).astype(np.float16))
        in_maps.append({
            "xT": xT, "wq": Wq, "wk": Wk, "wv": Wv, "wg": Wg, "wo": Wo,
            "cos2": c2, "sin2": s2,
        })
    return in_maps


def kernel_traced(x, Wq, Wk, Wv, Wg, Wo, cos, sin, block_size, trace=False,
                  **run_kwargs):
    assert int(block_size) == BLK
    from concourse import bass_utils

    if "nc" not in _NC_CACHE:
        _NC_CACHE["nc"] = _build_nc()
    nc = _NC_CACHE["nc"]

    in_maps = host_prep(x, Wq, Wk, Wv, Wg, Wo, cos, sin)
    res = bass_utils.run_bass_kernel_spmd(
        nc, in_maps, core_ids=list(range(NCORES)), trace=trace, **run_kwargs)
    out_flat = np.empty((B * S, E), dtype=np.float32)
    for cix in range(NCORES):
        out_flat[cix * R:(cix + 1) * R] = res.results[cix]["outT"].T
    return out_flat.reshape(B, S, E), res


def kernel(x, Wq, Wk, Wv, Wg, Wo, cos, sin, block_size):
    return kernel_traced(x, Wq, Wk, Wv, Wg, Wo, cos, sin, block_size)[0]
